# revision 1
# baseline (speedup 1.0000x reference)
"""Trainium2 kernel for DisplacementVectorsASU (gnn_message_passing).

Sharding: edge-shard M=4M across 8 cores (500k each, padded to 503808 =
128*96*41 slots); the small frac_coords table is replicated per core.

Gather strategy: the HW descriptor-generation engine only honors one
offset per partition for plain indirect DMA (multi-offset offset tables
lower incorrectly), so per-edge row gathers go through dma_gather
(InstDMAGatherAnt, the production MoE gather): a host-built stride-4
overlapping block table T4 (25000 x 64 f32; entry j = frac4 rows
4j..4j+15, 256B — dma_gather requires 256B-multiple elements and int16
indices, idx>>2 < 25000 fits) is gathered per tile, then a 1-of-4
on-chip select using the low 2 index bits (host-provided 0/1 mask
planes) recovers the exact node row. dma_gather writes edge i of a tile
to partition i%128, slot i//128; the host pre-permutes all per-edge
arrays into that slot order and inverse-permutes the output.

Math per edge: out_t = symmops[:, :3, :] @ [out_frac, 1]; periodic wrap
x - floor(x) built from round-to-nearest (fused +/-1.5*2^23
tensor_scalar) and an is_lt mask; result = in_frac - (wrap + trans).
"""
import sys

sys.path.insert(0, "/opt/trn_rl_repo")

import numpy as np

N_NODES = 100_000
M_TOTAL = 4_000_000
N_CORES = 8
P = 128
K = 96                  # slots per partition per tile
TILE = P * K            # 12288 edges per tile
NT = 41                 # tiles per core
M_CORE = TILE * NT      # 503808 padded edges per core
NB = 25_000             # stride-4 block-table entries
RND = 12582912.0        # 1.5 * 2^23 fp32 rounding constant

_cache = {}


def emit_tile(nc, pool, t, t4_d, iw0_d, iw1_d, mk_d, symm_d, tr_d, out_d):
    """Emit one 12288-edge tile: loads, 2 block-gathers, selects, math."""
    import concourse.mybir as mybir

    f32, i16 = mybir.dt.float32, mybir.dt.int16
    op = mybir.AluOpType

    iw0 = pool.tile((P, TILE // 16), i16, name="iw0")
    iw1 = pool.tile((P, TILE // 16), i16, name="iw1")
    mk = pool.tile((P, 4, K, 1), f32, name="mk")
    c0 = pool.tile((P, K, 64), f32, name="c0")
    c1 = pool.tile((P, K, 64), f32, name="c1")
    s = pool.tile((P, K, 16), f32, name="s")
    tr = pool.tile((P, K, 3), f32, name="tr")
    g0 = pool.tile((P, K, 4), f32, name="g0")
    g1 = pool.tile((P, K, 4), f32, name="g1")
    ta = pool.tile((P, K, 4), f32, name="ta")
    tb = pool.tile((P, K, 4), f32, name="tb")
    acc = pool.tile((P, K, 3), f32, name="acc")
    tmp = pool.tile((P, K, 3), f32, name="tmp")
    y = pool.tile((P, K, 3), f32, name="y")
    w = pool.tile((P, K, 3), f32, name="w")
    msk = pool.tile((P, K, 3), f32, name="msk")
    res = pool.tile((P, K, 3), f32, name="res")

    nc.sync.dma_start(iw0[:], iw0_d[t])
    nc.sync.dma_start(iw1[:], iw1_d[t])
    nc.sync.dma_start(mk[:].rearrange("p a k c -> p (a k c)"), mk_d[t])
    nc.sync.dma_start(s[:].rearrange("p k c -> p (k c)"), symm_d[t])
    nc.sync.dma_start(tr[:].rearrange("p k c -> p (k c)"), tr_d[t])
    nc.gpsimd.dma_gather(
        out_ap=c0[:], in_ap=t4_d[:], idxs_ap=iw0[:],
        num_idxs=TILE, num_idxs_reg=TILE, elem_size=64, single_packet=False)
    nc.gpsimd.dma_gather(
        out_ap=c1[:], in_ap=t4_d[:], idxs_ap=iw1[:],
        num_idxs=TILE, num_idxs_reg=TILE, elem_size=64, single_packet=False)

    v = nc.vector

    def select(dst, cand, mlo, mhi):
        # dst = cand[row lo], lo = mlo + 2*mhi in 0..3
        C = lambda r: cand[:, :, 4 * r:4 * r + 4]
        ML = mlo.to_broadcast((P, K, 4))
        MH = mhi.to_broadcast((P, K, 4))
        v.tensor_tensor(out=ta[:], in0=C(1), in1=C(0), op=op.subtract)
        v.tensor_tensor(out=ta[:], in0=ta[:], in1=ML, op=op.mult)
        v.tensor_tensor(out=ta[:], in0=ta[:], in1=C(0), op=op.add)
        v.tensor_tensor(out=tb[:], in0=C(3), in1=C(2), op=op.subtract)
        v.tensor_tensor(out=tb[:], in0=tb[:], in1=ML, op=op.mult)
        v.tensor_tensor(out=tb[:], in0=tb[:], in1=C(2), op=op.add)
        v.tensor_tensor(out=dst[:], in0=tb[:], in1=ta[:], op=op.subtract)
        v.tensor_tensor(out=dst[:], in0=dst[:], in1=MH, op=op.mult)
        v.tensor_tensor(out=dst[:], in0=dst[:], in1=ta[:], op=op.add)

    select(g0, c0, mk[:, 0], mk[:, 1])
    select(g1, c1, mk[:, 2], mk[:, 3])

    s4 = s[:].rearrange("p k (c j) -> p k c j", c=4)
    A = lambda j: s4[:, :, 0:3, j]
    G = lambda j: g1[:, :, j:j + 1].to_broadcast((P, K, 3))
    v.tensor_tensor(out=acc[:], in0=A(0), in1=G(0), op=op.mult)
    v.tensor_tensor(out=acc[:], in0=acc[:], in1=A(3), op=op.add)
    v.tensor_tensor(out=tmp[:], in0=A(1), in1=G(1), op=op.mult)
    v.tensor_tensor(out=acc[:], in0=acc[:], in1=tmp[:], op=op.add)
    v.tensor_tensor(out=tmp[:], in0=A(2), in1=G(2), op=op.mult)
    v.tensor_tensor(out=acc[:], in0=acc[:], in1=tmp[:], op=op.add)
    v.tensor_scalar(out=y[:], in0=acc[:], scalar1=RND, scalar2=-RND,
                    op0=op.add, op1=op.add)
    v.tensor_tensor(out=w[:], in0=acc[:], in1=y[:], op=op.subtract)
    v.tensor_scalar(out=msk[:], in0=w[:], scalar1=0.0, scalar2=None,
                    op0=op.is_lt)
    v.tensor_tensor(out=w[:], in0=w[:], in1=msk[:], op=op.add)
    v.tensor_tensor(out=w[:], in0=w[:], in1=tr[:], op=op.add)
    v.tensor_tensor(out=res[:], in0=g0[:, :, 0:3], in1=w[:], op=op.subtract)
    nc.sync.dma_start(out_d[t], res[:].rearrange("p k c -> p (k c)"))


def _build():
    if "nc" in _cache:
        return _cache["nc"]
    import concourse.mybir as mybir
    import concourse.tile as tile
    from concourse import bacc

    f32, i16 = mybir.dt.float32, mybir.dt.int16
    nc = bacc.Bacc(None, target_bir_lowering=False, debug=False)

    t4_d = nc.dram_tensor("t4", (NB, 64), f32, kind="ExternalInput")
    iw0_d = nc.dram_tensor("iw0", (NT, P, TILE // 16), i16, kind="ExternalInput")
    iw1_d = nc.dram_tensor("iw1", (NT, P, TILE // 16), i16, kind="ExternalInput")
    mk_d = nc.dram_tensor("mk", (NT, P, 4 * K), f32, kind="ExternalInput")
    symm_d = nc.dram_tensor("symm", (NT, P, K * 16), f32, kind="ExternalInput")
    tr_d = nc.dram_tensor("tr", (NT, P, K * 3), f32, kind="ExternalInput")
    out_d = nc.dram_tensor("out", (NT, P, K * 3), f32, kind="ExternalOutput")

    with tile.TileContext(nc) as tc:
        with tc.tile_pool(name="pool", bufs=2) as pool:
            for t in range(NT):
                emit_tile(nc, pool, t, t4_d, iw0_d, iw1_d, mk_d,
                          symm_d, tr_d, out_d)
    nc.compile()
    _cache["nc"] = nc
    return nc


def _prep(frac_coords, edge_indices, symmops, cell_translations):
    frac = np.asarray(frac_coords, np.float32)
    table = np.concatenate([frac, np.ones((N_NODES, 1), np.float32)], axis=1)
    # stride-4 overlapping blocks: T4[j] = frac4 rows 4j..4j+15
    flat = np.concatenate([table, np.zeros((16, 4), np.float32)]).ravel()
    T4 = np.ascontiguousarray(
        np.lib.stride_tricks.sliding_window_view(flat, 64)[::16][:NB])

    M_pad = M_CORE * N_CORES
    idx = np.zeros((2, M_pad), np.int32)
    idx[:, :M_TOTAL] = edge_indices
    symm = np.zeros((M_pad, 16), np.float32)
    symm[:M_TOTAL] = np.asarray(symmops, np.float32).reshape(M_TOTAL, 16)
    tr = np.zeros((M_pad, 3), np.float32)
    tr[:M_TOTAL] = cell_translations

    in_maps = []
    for c in range(N_CORES):
        sl = slice(c * M_CORE, (c + 1) * M_CORE)
        idc = idx[:, sl]
        # wrapped-16 int16 block indices: index i at [16g + i%16, i//16]
        i4 = (idc >> 2).astype(np.int16).reshape(2, NT, TILE // 16, 16)
        iw = np.tile(i4.transpose(0, 1, 3, 2), (1, 1, 8, 1))
        # low-bit 0/1 mask planes in slot order: edge i -> (i%128, i//128)
        lo = (idc & 3).reshape(2, NT, K, P)
        mk = np.empty((NT, P, 4, K), np.float32)
        mk[:, :, 0] = (lo[0] & 1).transpose(0, 2, 1)
        mk[:, :, 1] = (lo[0] >> 1).transpose(0, 2, 1)
        mk[:, :, 2] = (lo[1] & 1).transpose(0, 2, 1)
        mk[:, :, 3] = (lo[1] >> 1).transpose(0, 2, 1)
        # per-edge arrays into slot order
        sm = symm[sl].reshape(NT, K, P, 16).transpose(0, 2, 1, 3)
        trc = tr[sl].reshape(NT, K, P, 3).transpose(0, 2, 1, 3)
        in_maps.append({
            "t4": T4,
            "iw0": np.ascontiguousarray(iw[0]),
            "iw1": np.ascontiguousarray(iw[1]),
            "mk": np.ascontiguousarray(mk.reshape(NT, P, 4 * K)),
            "symm": np.ascontiguousarray(sm.reshape(NT, P, K * 16)),
            "tr": np.ascontiguousarray(trc.reshape(NT, P, K * 3)),
        })
    return in_maps


def kernel(frac_coords, edge_indices, symmops, cell_translations):
    from concourse.bass_utils import run_bass_kernel_spmd
    nc = _build()
    in_maps = _prep(frac_coords, edge_indices, symmops, cell_translations)
    res = run_bass_kernel_spmd(nc, in_maps, list(range(N_CORES)))
    # device output is slot order; slot (p, k) holds edge k*128 + p
    outs = []
    for c in range(N_CORES):
        o = res.results[c]["out"].reshape(NT, P, K, 3)
        outs.append(o.transpose(0, 2, 1, 3).reshape(M_CORE, 3))
    return np.concatenate(outs, axis=0)[:M_TOTAL]



# revision 3
# speedup vs baseline: 2.4312x; 2.4312x over previous
"""Trainium2 kernel v3 for DisplacementVectorsASU — two-pass, both sides
sorted, no per-edge descriptors.

The swdge descriptor-generation rate (~8.7ns/desc, single queue) made the
random idx0 per-edge gather the wall (12288 desc/tile). v3 eliminates it:

- Pass A: edges sorted by idx1, packed in 32-edge sub-runs with idx1 in a
  16-row window -> ONE descriptor per sub-run (384/tile). Computes
  W = wrap(A @ [out_frac,1]) + translation and stores it to DRAM in
  A-slot order.
- Host: permutes W from A-slot order to B-slot order (pure per-edge data
  movement, same class as the existing pre/post slot permutes).
- Pass B: edges sorted by idx0, same sub-run structure -> in_frac via run
  descriptors; streams W sequentially; result = in_frac - W.

Both passes are ~384 descs + ~2MB streams + ~40 DVE ops per tile; the
descriptor stream drops from 12672 to 384 per tile per pass.
"""
import sys

sys.path.insert(0, "/opt/trn_rl_repo")

import numpy as np

N_NODES = 100_000
M_TOTAL = 4_000_000
N_CORES = 8
M_EDGE_CORE = M_TOTAL // N_CORES
P = 128
K = 96
TILE = P * K
NT = 41
RUNW = 32
NRUN = TILE // RUNW     # 384
NBK = 25_000
RND = 12582912.0

_cache = {}


def _sel16(nc, pool, g, cb, lu, NS):
    """g[:, k, :] = cb[:, k//RUNW, 4*l:4*l+3] where l = lu[:, k] (0..15)."""
    import concourse.mybir as mybir
    op = mybir.AluOpType
    v = nc.vector
    m1 = pool.tile((P, K, 1), mybir.dt.uint8, name="m1")
    gv = g[:].rearrange("p (s u) c -> p s u c", s=NS)
    B = lambda ap: ap.rearrange("p s (u c) -> p s u c", u=1) \
        .to_broadcast((P, NS, RUNW, 3))
    v.tensor_copy(out=gv, in_=B(cb[:, :, 0:3]))
    m1v = m1[:].rearrange("p (s u) c -> p s u c", s=NS) \
        .to_broadcast((P, NS, RUNW, 3))
    for l in range(1, 16):
        v.tensor_scalar(out=m1[:], in0=lu[:], scalar1=l, scalar2=None,
                        op0=op.is_equal)
        v.copy_predicated(gv, m1v, B(cb[:, :, 4 * l:4 * l + 3]))


def emit_tile_a(nc, pool, t, t4_d, iwb_d, l1u_d, symm_d, tr_d, w_d):
    """Pass A tile: out_frac run-gather + select, affine + wrap, store W."""
    import concourse.mybir as mybir
    f32, i16, u8 = mybir.dt.float32, mybir.dt.int16, mybir.dt.uint8
    op = mybir.AluOpType
    NS = NRUN // P

    iwb = pool.tile((P, NRUN // 16), i16, name="iwb")
    l1u = pool.tile((P, K, 1), u8, name="l1u")
    cb = pool.tile((P, NS, 64), f32, name="cb")
    s = pool.tile((P, K, 12), f32, name="s")
    tr = pool.tile((P, K, 3), f32, name="tr")
    g1 = pool.tile((P, K, 3), f32, name="g1")
    acc = pool.tile((P, K, 3), f32, name="acc")
    tmp = pool.tile((P, K, 3), f32, name="tmp")
    y = pool.tile((P, K, 3), f32, name="y")
    w = pool.tile((P, K, 3), f32, name="w")
    msk = pool.tile((P, K, 3), f32, name="msk")

    nc.sync.dma_start(iwb[:], iwb_d[t])
    nc.sync.dma_start(l1u[:].rearrange("p k c -> p (k c)"), l1u_d[t])
    nc.sync.dma_start(s[:].rearrange("p k c -> p (k c)"), symm_d[t])
    nc.sync.dma_start(tr[:].rearrange("p k c -> p (k c)"), tr_d[t])
    nc.gpsimd.dma_gather(
        out_ap=cb[:], in_ap=t4_d[:], idxs_ap=iwb[:],
        num_idxs=NRUN, num_idxs_reg=NRUN, elem_size=64, single_packet=False)

    v = nc.vector
    _sel16(nc, pool, g1, cb, l1u, NS)
    s4 = s[:].rearrange("p k (c j) -> p k c j", c=3)
    A = lambda j: s4[:, :, :, j]
    G = lambda j: g1[:, :, j:j + 1].to_broadcast((P, K, 3))
    v.tensor_tensor(out=acc[:], in0=A(0), in1=G(0), op=op.mult)
    v.tensor_tensor(out=acc[:], in0=acc[:], in1=A(3), op=op.add)
    v.tensor_tensor(out=tmp[:], in0=A(1), in1=G(1), op=op.mult)
    v.tensor_tensor(out=acc[:], in0=acc[:], in1=tmp[:], op=op.add)
    v.tensor_tensor(out=tmp[:], in0=A(2), in1=G(2), op=op.mult)
    v.tensor_tensor(out=acc[:], in0=acc[:], in1=tmp[:], op=op.add)
    v.tensor_scalar(out=y[:], in0=acc[:], scalar1=RND, scalar2=-RND,
                    op0=op.add, op1=op.add)
    v.tensor_tensor(out=w[:], in0=acc[:], in1=y[:], op=op.subtract)
    v.tensor_scalar(out=msk[:], in0=w[:], scalar1=0.0, scalar2=None,
                    op0=op.is_lt)
    v.tensor_tensor(out=w[:], in0=w[:], in1=msk[:], op=op.add)
    v.tensor_tensor(out=w[:], in0=w[:], in1=tr[:], op=op.add)
    nc.sync.dma_start(w_d[t], w[:].rearrange("p k c -> p (k c)"))


def emit_tile_b(nc, pool, t, t4_d, iwb_d, l0u_d, wb_d, out_d):
    """Pass B tile: in_frac run-gather + select, subtract streamed W."""
    import concourse.mybir as mybir
    f32, i16, u8 = mybir.dt.float32, mybir.dt.int16, mybir.dt.uint8
    op = mybir.AluOpType
    NS = NRUN // P

    iwb = pool.tile((P, NRUN // 16), i16, name="iwb")
    l0u = pool.tile((P, K, 1), u8, name="l0u")
    cb = pool.tile((P, NS, 64), f32, name="cb")
    wb = pool.tile((P, K, 3), f32, name="wb")
    g0 = pool.tile((P, K, 3), f32, name="g0")
    res = pool.tile((P, K, 3), f32, name="res")

    nc.sync.dma_start(iwb[:], iwb_d[t])
    nc.sync.dma_start(l0u[:].rearrange("p k c -> p (k c)"), l0u_d[t])
    nc.sync.dma_start(wb[:].rearrange("p k c -> p (k c)"), wb_d[t])
    nc.gpsimd.dma_gather(
        out_ap=cb[:], in_ap=t4_d[:], idxs_ap=iwb[:],
        num_idxs=NRUN, num_idxs_reg=NRUN, elem_size=64, single_packet=False)

    v = nc.vector
    _sel16(nc, pool, g0, cb, l0u, NS)
    v.tensor_tensor(out=res[:], in0=g0[:], in1=wb[:], op=op.subtract)
    nc.sync.dma_start(out_d[t], res[:].rearrange("p k c -> p (k c)"))


def _build():
    if "ab" in _cache:
        return _cache["ab"]
    import concourse.mybir as mybir
    import concourse.tile as tile
    from concourse import bacc

    f32, i16, u8 = mybir.dt.float32, mybir.dt.int16, mybir.dt.uint8

    ncA = bacc.Bacc(None, target_bir_lowering=False, debug=False)
    t4_d = ncA.dram_tensor("t4", (NBK, 64), f32, kind="ExternalInput")
    iwb_d = ncA.dram_tensor("iwb", (NT, P, NRUN // 16), i16, kind="ExternalInput")
    l1u_d = ncA.dram_tensor("l1u", (NT, P, K), u8, kind="ExternalInput")
    symm_d = ncA.dram_tensor("symm", (NT, P, K * 12), f32, kind="ExternalInput")
    tr_d = ncA.dram_tensor("tr", (NT, P, K * 3), f32, kind="ExternalInput")
    w_d = ncA.dram_tensor("w", (NT, P, K * 3), f32, kind="ExternalOutput")
    with tile.TileContext(ncA) as tc:
        with tc.tile_pool(name="pool", bufs=2) as pool:
            for t in range(NT):
                emit_tile_a(ncA, pool, t, t4_d, iwb_d, l1u_d, symm_d, tr_d, w_d)
    ncA.compile()

    ncB = bacc.Bacc(None, target_bir_lowering=False, debug=False)
    t4b_d = ncB.dram_tensor("t4", (NBK, 64), f32, kind="ExternalInput")
    iwb0_d = ncB.dram_tensor("iwb0", (NT, P, NRUN // 16), i16, kind="ExternalInput")
    l0u_d = ncB.dram_tensor("l0u", (NT, P, K), u8, kind="ExternalInput")
    wb_d = ncB.dram_tensor("wb", (NT, P, K * 3), f32, kind="ExternalInput")
    out_d = ncB.dram_tensor("out", (NT, P, K * 3), f32, kind="ExternalOutput")
    with tile.TileContext(ncB) as tc:
        with tc.tile_pool(name="pool", bufs=2) as pool:
            for t in range(NT):
                emit_tile_b(ncB, pool, t, t4b_d, iwb0_d, l0u_d, wb_d, out_d)
    ncB.compile()

    _cache["ab"] = (ncA, ncB)
    return _cache["ab"]


def _wrap16(lst):
    n = len(lst)
    a = lst.reshape(n // 16, 16).T
    return np.ascontiguousarray(np.tile(a, (8, 1)).astype(np.int16))


def _pack(idx):
    """Sort idx, pack RUNW-edge sub-runs with values in [4b, 4b+16).

    Returns (eslot, l_local, blk): eslot[e] = slot of edge e,
    l_local/blk in slot order."""
    n = idx.shape[0]
    order = np.argsort(idx, kind="stable").astype(np.int64)
    s1 = idx[order]
    run_blk = []
    pos = 0
    starts = np.empty(n + 1, np.int64)
    nrun = 0
    while pos < n:
        b = s1[pos] >> 2
        hi = 4 * b + 16
        end = pos + int(np.searchsorted(s1[pos:pos + RUNW + 200], hi,
                                        side="left"))
        end = min(end, pos + RUNW)
        starts[nrun] = pos
        run_blk.append(b)
        nrun += 1
        pos = end
    starts[nrun] = n
    assert nrun <= NT * NRUN, f"packing overflow: {nrun} runs"
    R = np.repeat(np.arange(nrun, dtype=np.int64), np.diff(starts[:nrun + 1]))
    u = np.arange(n, dtype=np.int64) - starts[R]
    tt = R // NRUN
    j = R % NRUN
    slot = tt * TILE + (j % P) * K + (j // P) * RUNW + u
    blk = np.zeros(NT * NRUN, np.int64)
    blk[:nrun] = np.array(run_blk, np.int64)
    nslot = NT * TILE
    lloc = np.zeros(nslot, np.int64)
    lloc[slot] = s1 - 4 * blk[R]
    assert lloc.min() >= 0 and lloc.max() < 16
    eslot = np.empty(n, np.int64)
    eslot[order] = slot
    return eslot, lloc, blk


def _layout(blk, lloc):
    iwb = np.stack([_wrap16(blk[t * NRUN:(t + 1) * NRUN]) for t in range(NT)])
    lu = lloc.reshape(NT, P, K).astype(np.uint8)
    return np.ascontiguousarray(iwb), np.ascontiguousarray(lu)


def _prep(frac_coords, edge_indices, symmops, cell_translations):
    frac = np.asarray(frac_coords, np.float32)
    table = np.concatenate([frac, np.ones((N_NODES, 1), np.float32)], axis=1)
    flat = np.concatenate([table, np.zeros((16, 4), np.float32)]).ravel()
    t4 = np.ascontiguousarray(
        np.lib.stride_tricks.sliding_window_view(flat, 64)[::16][:NBK])

    ei = np.asarray(edge_indices, np.int64)
    symm12 = np.asarray(symmops, np.float32).reshape(M_TOTAL, 4, 4)[:, :3, :] \
        .reshape(M_TOTAL, 12)
    trf = np.asarray(cell_translations, np.float32)

    a_maps, b_maps, eslots = [], [], []
    nslot = NT * TILE
    for c in range(N_CORES):
        sl = slice(c * M_EDGE_CORE, (c + 1) * M_EDGE_CORE)
        idx0, idx1 = ei[0, sl], ei[1, sl]
        eslotA, l1loc, blkA = _pack(idx1)
        eslotB, l0loc, blkB = _pack(idx0)
        iwbA, l1u = _layout(blkA, l1loc)
        iwbB, l0u = _layout(blkB, l0loc)
        sm = np.zeros((nslot, 12), np.float32)
        trc = np.zeros((nslot, 3), np.float32)
        sm[eslotA] = symm12[sl]
        trc[eslotA] = trf[sl]
        a_maps.append({
            "t4": t4, "iwb": iwbA, "l1u": l1u,
            "symm": np.ascontiguousarray(sm.reshape(NT, P, K * 12)),
            "tr": np.ascontiguousarray(trc.reshape(NT, P, K * 3)),
        })
        b_maps.append({"t4": t4, "iwb0": iwbB, "l0u": l0u})
        eslots.append((eslotA, eslotB))
    return a_maps, b_maps, eslots


def kernel(frac_coords, edge_indices, symmops, cell_translations):
    from concourse.bass_utils import run_bass_kernel_spmd
    ncA, ncB = _build()
    a_maps, b_maps, eslots = _prep(frac_coords, edge_indices, symmops,
                                   cell_translations)
    resA = run_bass_kernel_spmd(ncA, a_maps, list(range(N_CORES)))
    nslot = NT * TILE
    for c in range(N_CORES):
        WA = resA.results[c]["w"].reshape(nslot, 3)
        eslotA, eslotB = eslots[c]
        WB = np.zeros((nslot, 3), np.float32)
        WB[eslotB] = WA[eslotA]
        b_maps[c]["wb"] = np.ascontiguousarray(WB.reshape(NT, P, K * 3))
    resB = run_bass_kernel_spmd(ncB, b_maps, list(range(N_CORES)))
    out = np.empty((M_TOTAL, 3), np.float32)
    for c in range(N_CORES):
        o = resB.results[c]["out"].reshape(nslot, 3)
        out[c * M_EDGE_CORE:(c + 1) * M_EDGE_CORE] = o[eslots[c][1]]
    return out
